# revision 1
# baseline (speedup 1.0000x reference)
"""Trainium2 Bass kernel for masked cross-attention (sparse_attention).

Reference computation (per batch b):
    q = x @ Wq + bq                      # [N, hd]   (hd = 8 heads * 32)
    k = ctx @ Wk + bk ; v = ctx @ Wv + bv
    dots[h,i,j] = q_h[i] . k_h[j]  + frag_mask[j]   (masked j -> -inf)
    attn = softmax_j(dots) ; out = (attn @ v) @ W_out + b_out

Distribution: 8 cores = 4 batches x 2 head-groups (4 heads each).
Host-side prep: compact context along j by the boolean mask (~50% kept),
transpose x/context to [dim, tokens] layout, slice weights per head group.

Device per core:
  - Loads are fused into few single-dma_start transfers (the HWDGE
    costs ~5ns/descriptor serially, so many small loads serialize the
    pipeline head): wq+wk as one [128,512] tile, x^T packed per-i-half
    so the Q projection starts after half the load, a small first cT
    chunk so the K projection unblocks early, bq+bk as one [128,2].
    All transfers are contiguous per DRAM row (strided multi-run DMAs
    measured ~10us slower on HW).  Bulk cT/wv/wo ride the gpsimd SWDGE
    ring in parallel with the sync-ring critical chain.
  - QKV projections on PE in fp16 (the full-rate deterministic 16-bit
    path -- float32r matmuls showed nondeterministic drift).  q/k
    stored fp16, v stored bfloat16.
  - S^T = K.QT per head: 32-row-tiled fp16 matmuls; exp on ACT with a
    per-partition additive-mask bias (no max subtraction; |logits| <=
    ~40 so fp32 exp is overflow-safe); output bf16.
  - P.V and the softmax denominators via column-tiled bf16 matmuls
    accumulated in PSUM across j-tiles (denominator uses an all-ones
    [128,32] stationary so it lands broadcast across each head's 32
    partitions, partition-aligned with PV for the normalize).
    Accumulators are DVE-memset to zero and all matmuls use
    start=False: no whole-bank has_written clears racing the col-tiled
    sibling writes.
  - normalize with DVE reciprocal+mul, project with W_out (v-bias and
    b_out are folded into the host-side output assembly, exactly).
"""

import numpy as np
import ml_dtypes

import concourse.bass as bass  # noqa: F401
import concourse.mybir as mybir
import concourse.tile as tile
import concourse.bacc as bacc
from concourse.bass_utils import run_bass_kernel_spmd

F32 = mybir.dt.float32
F32R = mybir.dt.float32r
F16 = mybir.dt.float16
BF16 = mybir.dt.bfloat16
AF = mybir.ActivationFunctionType

B = 4
N_Q = 1024          # queries per batch
DIM = 256           # model dim
D_HEAD = 32
HPC = 4             # heads per core
HD = 128            # HPC * D_HEAD: head-group width
NEG = -60000.0      # additive mask for dropped/padded j (exp -> exactly 0)

_cache: dict = {}
last_results = None  # test.py introspection


def _build(mjt: int, reps: int = 1, debug: bool = False):
    """Build + compile the per-core Bass program for mjt j-tiles of 128.

    reps>1 replicates the whole body serially (bench slope timing only).
    debug adds intermediate-dump outputs (diagnosis only).
    """
    mp = mjt * 128
    nc = bacc.Bacc("TRN2", target_bir_lowering=False, debug=False)

    d_xT = nc.declare_dram_parameter("xT", [2, 128, N_Q], F16, isOutput=False)
    d_cT = nc.declare_dram_parameter("cT", [2, 128, mp], F16, isOutput=False)
    d_wqk = nc.declare_dram_parameter("wqk", [128, 4 * HD], F16, isOutput=False)
    d_wv = nc.declare_dram_parameter("wv", [128, 2 * HD], F16, isOutput=False)
    d_wo = nc.declare_dram_parameter("wo", [128, DIM], F16, isOutput=False)
    d_bqk = nc.declare_dram_parameter("bqk", [128, 2], F32, isOutput=False)
    d_am = nc.declare_dram_parameter("amask", [128, mjt], F32, isOutput=False)
    d_out = nc.declare_dram_parameter("outT", [2, 128, N_Q], F16, isOutput=True)
    if debug:
        d_dbg = {
            nm: nc.declare_dram_parameter(f"dbg_{nm}", [128, width], F32, isOutput=True)
            for nm, width in [("l", N_Q), ("pv", N_Q), ("q", N_Q), ("k", mp), ("at", N_Q)]
        }

    with tile.TileContext(nc) as tc:
        with (
            tc.tile_pool(name="pin", bufs=1) as pin,
            tc.tile_pool(name="pwork", bufs=1) as pwork,
            tc.tile_pool(name="pe", bufs=10) as pe_pool,
            tc.tile_pool(name="ps_s", bufs=2, space="PSUM") as ps_s,
            tc.tile_pool(name="ps_acc", bufs=1, space="PSUM") as ps_acc,
        ):
          for _rep in range(reps):
            # ---- loads (Q-projection deps first: they gate the PE ramp) ----
            xT_f = pin.tile([128, 2 * N_Q], F16)
            wqk_f = pin.tile([128, 4 * HD], F16)
            wv_f = pin.tile([128, 2 * HD], F16)
            bqk_f = pin.tile([128, 2], F32)
            # xT_f columns: ih*1024 + ct*512 + i (per-ih packed halves)
            wq = [wqk_f[:, i * HD:(i + 1) * HD] for i in range(2)]
            wk = [wqk_f[:, (2 + i) * HD:(3 + i) * HD] for i in range(2)]
            wv = [wv_f[:, i * HD:(i + 1) * HD] for i in range(2)]
            bq = bqk_f[:, 0:1]
            bk = bqk_f[:, 1:2]
            cT_f = pin.tile([128, 2 * mp], F16)
            cT = [cT_f[:, i * mp:(i + 1) * mp] for i in range(2)]
            mh = (mp + 1) // 2
            am = pin.tile([128, mjt], F32)
            nc.sync.dma_start(wqk_f[:], d_wqk[:])
            # xT first half immediately after the weights: Q gates the PE
            # ramp and the first S; the small cT chunks for K follow
            nc.sync.dma_start(xT_f[:, 0:N_Q], d_xT[0])
            for ct in range(2):
                nc.sync.dma_start(cT_f[:, ct * mp:ct * mp + 256], d_cT[ct][:, 0:256])
            nc.sync.dma_start(bqk_f[:], d_bqk[:])
            nc.sync.dma_start(am[:], d_am[:])
            for ct in range(2):
                nc.sync.dma_start(
                    cT_f[:, ct * mp + 256:ct * mp + mh], d_cT[ct][:, 256:mh])
            ones = pin.tile([128, D_HEAD], BF16)
            nc.vector.memset(ones[:], 1.0)
            # xT second half rides the otherwise-idle gpsimd ring, in
            # parallel with the sync chain, so Q finishes ~1.5us earlier
            nc.gpsimd.dma_start(xT_f[:, N_Q:2 * N_Q], d_xT[1])
            nc.gpsimd.dma_start(wv_f[:], d_wv[:])
            wo = pin.tile([128, DIM], F16)
            nc.gpsimd.dma_start(wo[:], d_wo[:])
            for ct in range(2):
                nc.gpsimd.dma_start(cT_f[:, ct * mp + mh:(ct + 1) * mp], d_cT[ct][:, mh:mp])

            # ---- persistent SBUF working tensors ----
            # q/k in fp16 (11-bit mantissa): logit noise ~1.4e-3, on the
            # deterministic 16-bit PE path (float32r matmuls showed
            # nondeterministic drift when mixed into this program).
            qT_hi = pwork.tile([128, N_Q], F16)   # [head*dim, i]
            kT_hi = pwork.tile([128, mp], F16)    # [head*dim, j]
            vnat = pwork.tile([128, mp], BF16)    # [j_local, jt*128 + head*dim]
            attnT = pwork.tile([128, N_Q], F16)
            linv = pwork.tile([128, N_Q], F32)
            outT = [pwork.tile([128, N_Q], F16, tag=f"outT{i}", name=f"outT{i}")
                    for i in range(2)]

            # warm the ACT exp table set during the DMA phase
            warm = pwork.tile([128, 1], F32, tag="warm")
            nc.vector.memset(warm[:], 0.0)
            warm2 = pwork.tile([128, 1], F32, tag="warm2")
            nc.scalar.activation(warm2[:], warm[:], AF.Exp)

            # ---- persistent PSUM accumulators (explicitly zeroed) ----
            pv_acc = ps_acc.tile([128, N_Q], F32, tag="pv")
            l_acc = ps_acc.tile([128, N_Q], F32, tag="l")
            nc.vector.memset(pv_acc[:], 0.0)
            nc.vector.memset(l_acc[:], 0.0)

            # ---- Q^T projection: qT = Wq^T @ x^T (+bq) ----
            for ih in range(2):
                ps = ps_s.tile([128, N_Q], F32, tag="s")
                sl = slice(ih * 512, ih * 512 + 512)
                for ct in range(2):
                    x0 = ih * N_Q + ct * 512
                    nc.tensor.matmul(
                        ps[:, 0:512], wq[ct][:], xT_f[:, x0:x0 + 512],
                        start=(ct == 0), stop=(ct == 1),
                    )
                nc.vector.tensor_scalar_add(qT_hi[:, sl], ps[:, 0:512], bq)

            # ---- main loop over j-tiles (software-pipelined: PV/L of the
            # previous tile are emitted AFTER the current tile's S matmuls,
            # so the next exp on ACT is never blocked behind them) ----
            def emit_pv(j0_p, e_prev, last):
                for ih in range(2):
                    sl = slice(ih * 512, ih * 512 + 512)
                    for h in range(HPC):
                        nc.tensor.matmul(
                            pv_acc[32 * h:32 * h + 32, sl],
                            vnat[:, j0_p + 32 * h:j0_p + 32 * h + 32],
                            e_prev[h][:, sl],
                            start=False, stop=(last and h == HPC - 1),
                            tile_position=(0, 32 * h),
                            skip_group_check=True,
                        )
                    for h in range(HPC):
                        nc.tensor.matmul(
                            l_acc[32 * h:32 * h + 32, sl],
                            ones[:],
                            e_prev[h][:, sl],
                            start=False, stop=(last and h == HPC - 1),
                            tile_position=(0, 32 * h),
                            skip_group_check=True,
                        )

            prev = None  # (j0, e_tiles) of the previous j-tile
            for jt in range(mjt):
                # K^T projection, one j-tile at a time (spread evenly)
                j0 = jt * 128
                ps = ps_s.tile([128, N_Q], F32, tag="s")
                for ct in range(2):
                    nc.tensor.matmul(
                        ps[:, 0:128], wk[ct][:], cT[ct][:, j0:j0 + 128],
                        start=(ct == 0), stop=(ct == 1),
                    )
                nc.vector.tensor_scalar_add(kT_hi[:, j0:j0 + 128], ps[:, 0:128], bk)

                # V projection for this j-tile: [128 j, 128 hd]
                psv = ps_s.tile([128, N_Q], F32, tag="s")
                for ct in range(2):
                    nc.tensor.matmul(
                        psv[:, 0:HD], cT[ct][:, j0:j0 + 128], wv[ct][:],
                        start=(ct == 0), stop=(ct == 1),
                    )
                nc.vector.tensor_copy(vnat[:, j0:j0 + 128], psv[:, 0:HD])

                # S^T + exp per head
                e_tiles = []
                for h in range(HPC):
                    hp = slice(32 * h, 32 * h + 32)
                    s_ps = ps_s.tile([128, N_Q], F32, tag="s")
                    for ih in range(2):
                        sl = slice(ih * 512, ih * 512 + 512)
                        nc.tensor.matmul(
                            s_ps[:, sl],
                            kT_hi[hp, j0:j0 + 128],
                            qT_hi[hp, sl],
                            start=True, stop=True,
                            tile_position=(32 * h, 0),
                        )
                    e_t = pe_pool.tile([128, N_Q], BF16, tag="e")
                    nc.scalar.activation(
                        e_t[:], s_ps[:], AF.Exp, bias=am[:, jt:jt + 1],
                    )
                    e_tiles.append(e_t)

                # P @ V and row-sums for the PREVIOUS tile
                if prev is not None:
                    emit_pv(prev[0], prev[1], last=False)
                prev = (j0, e_tiles)
                if jt == mjt - 1:
                    # flush immediately: it hides under this tile's exps and
                    # shortens the post-exp tail
                    emit_pv(prev[0], prev[1], last=True)
                    prev = None


            # ---- debug dumps ----
            if debug:
                dbg_l_s = pwork.tile([128, N_Q], F32, tag="dbg_l_s")
                nc.vector.tensor_copy(dbg_l_s[:], l_acc[:])
                nc.sync.dma_start(d_dbg["l"][:], dbg_l_s[:])
                dbg_pv_s = pwork.tile([128, N_Q], F32, tag="dbg_pv_s")
                nc.vector.tensor_copy(dbg_pv_s[:], pv_acc[:])
                nc.sync.dma_start(d_dbg["pv"][:], dbg_pv_s[:])
                dbg_q_s = pwork.tile([128, N_Q], F32, tag="dbg_q_s")
                nc.vector.tensor_copy(dbg_q_s[:], qT_hi[:])
                nc.sync.dma_start(d_dbg["q"][:], dbg_q_s[:])
                dbg_k_s = pwork.tile([128, mp], F32, tag="dbg_k_s")
                nc.vector.tensor_copy(dbg_k_s[:], kT_hi[:])
                nc.sync.dma_start(d_dbg["k"][:], dbg_k_s[:])

            # ---- normalize + output projection, per i-half so the PE can
            # start projecting half 0 while the DVE still normalizes half 1
            for ih in range(2):
                sl = slice(ih * 512, ih * 512 + 512)
                nc.vector.reciprocal(linv[:, sl], l_acc[:, sl])
                nc.vector.tensor_tensor(
                    attnT[:, sl], pv_acc[:, sl], linv[:, sl], mybir.AluOpType.mult)
                for dt in range(2):
                    ps = ps_s.tile([128, N_Q], F32, tag="s")
                    nc.tensor.matmul(
                        ps[:, 0:512], wo[:, dt * 128:dt * 128 + 128], attnT[:, sl],
                        start=True, stop=True,
                    )
                    # copy on ACT (idle after the last exp): the DVE can
                    # proceed with half 1's reciprocal+mul while ACT drains
                    # half 0's output
                    nc.scalar.copy(outT[dt][:, sl], ps[:, 0:512])
                    nc.sync.dma_start(d_out[dt][:, sl], outT[dt][:, sl])
            if debug:
                nc.sync.dma_start(d_dbg["at"][:], attnT[:])

    nc.compile()
    return nc


def build_in_maps(inputs, keeps, mjt):
    x = np.ascontiguousarray(np.asarray(inputs["x"], dtype=np.float32))
    context = np.ascontiguousarray(np.asarray(inputs["context"], dtype=np.float32))
    frag_mask = np.asarray(inputs["frag_mask"], dtype=np.float32)
    W_qkv = np.ascontiguousarray(np.asarray(inputs["W_qkv"], dtype=np.float32))
    b_qkv = np.asarray(inputs["b_qkv"], dtype=np.float32)
    W_out = np.ascontiguousarray(np.asarray(inputs["W_out"], dtype=np.float32))
    mp = mjt * 128
    in_maps = []
    for core in range(8):
        b, hh = core % B, core // B
        keep = keeps[b]
        cnt = len(keep)
        cT = np.zeros((DIM, mp), dtype=np.float32)
        cT[:, :cnt] = context[b][keep].T
        amask = np.full((mp,), NEG, dtype=np.float32)
        amask[:cnt] = frag_mask[b][keep]
        hs = slice(hh * HD, (hh + 1) * HD)
        wq2 = W_qkv[:, hs].reshape(2, 128, HD)
        wk2 = W_qkv[:, 256:512][:, hs].reshape(2, 128, HD)
        wv2 = W_qkv[:, 512:768][:, hs].reshape(2, 128, HD)
        xr = x[b].T.reshape(2, 128, N_Q)
        xih = np.stack([
            np.concatenate([xr[0][:, 0:512], xr[1][:, 0:512]], axis=1),
            np.concatenate([xr[0][:, 512:1024], xr[1][:, 512:1024]], axis=1),
        ])
        in_maps.append({
            "xT": np.ascontiguousarray(xih).astype(np.float16),
            "cT": np.ascontiguousarray(cT.reshape(2, 128, mp)).astype(np.float16),
            "wqk": np.ascontiguousarray(
                np.concatenate([wq2[0], wq2[1], wk2[0], wk2[1]], axis=1)
            ).astype(np.float16),
            "wv": np.ascontiguousarray(
                np.concatenate([wv2[0], wv2[1]], axis=1)).astype(np.float16),
            "wo": np.ascontiguousarray(W_out[hs, :]).astype(np.float16),
            "bqk": np.ascontiguousarray(
                np.stack([b_qkv[0:256][hs], b_qkv[256:512][hs]], axis=1)),
            "amask": np.ascontiguousarray(amask.reshape(mjt, 128).T),
        })
    return in_maps


def kernel(x, context, mask, frag_mask, W_qkv, b_qkv, W_out, b_out):
    global last_results
    mask = np.asarray(mask).astype(bool)
    b_out = np.asarray(b_out, dtype=np.float32)

    keeps = [np.nonzero(mask[b])[0] for b in range(B)]
    mjt = max(1, max((len(k) + 127) // 128 for k in keeps))

    key = (mjt, 1, False)
    if key not in _cache:
        _cache[key] = _build(mjt)
    nc = _cache[key]

    inputs = {"x": x, "context": context, "frag_mask": frag_mask,
              "W_qkv": W_qkv, "b_qkv": b_qkv, "W_out": W_out}
    in_maps = build_in_maps(inputs, keeps, mjt)

    res = run_bass_kernel_spmd(nc, in_maps, list(range(8)))
    last_results = res

    out = np.zeros((B, N_Q, DIM), dtype=np.float32)
    for core in range(8):
        b = core % B
        partial = res.results[core]["outT"].astype(np.float32).reshape(DIM, N_Q)
        out[b] += partial.T
    b_qkv = np.asarray(b_qkv, dtype=np.float32)
    out += (b_out + b_qkv[512:768] @ np.asarray(W_out, dtype=np.float32))[None, None, :]
    return out

